# revision 22
# baseline (speedup 1.0000x reference)
"""BitBottleneck (ternary-quantized 3x3 conv x2 + BN + SiLU + residual) on 8 trn2 cores.

Strategy:
  - Data-parallel over batch: 32 images -> 4 per core, no collectives.
  - Ternary quantization is folded on host: w_q = s*t with t in {-1,0,+1};
    conv(x, s*t) == s*conv(x, t), and s is folded into the BN scale. The
    weights the PE sees are exactly representable, so matmul precision is
    limited only by the activations (float32r ~ tf32-ish rounding).
  - Each conv is an implicit GEMM: for each 8-row output block (N=448),
    accumulate 9 taps x cin-blocks of [K=128, M=128] x [K=128, N=448]
    matmuls into one PSUM bank. Inputs live in SBUF zero-padded to 58x58
    so every tap is a pure strided view.
  - ScalarE applies BN+SiLU (scale/bias per partition channel) reading PSUM,
    writing SBUF. VectorE adds the residual. fp32 storage everywhere;
    matmul operands bitcast to float32r (1 cycle/row instead of fp32's 4).
"""

import sys

if "/opt/trn_rl_repo" not in sys.path:
    sys.path.insert(0, "/opt/trn_rl_repo")

import numpy as np

B, C, H, W = 32, 256, 56, 56
HID = 128
NCORES = 8
IPC = B // NCORES  # images per core
HP, WP = H + 2, W + 2  # padded 58x58
PADN = HP * WP  # 3364
RB = 8  # output rows per row-block
NRB = H // RB  # 7
NMM = RB * W  # 448 matmul free dim
BN_EPS = 1e-5
Q_EPS = 1e-5

import os

MM_DT = os.environ.get("KMMDT", "f16")  # "f32r", "bf16", or "f16"

_CACHE = {}


def _build_nc():
    if "nc" in _CACHE:
        return _CACHE["nc"]

    import concourse.bass as bass
    import concourse.mybir as mybir
    import concourse.tile as tile
    from concourse import bacc

    f32 = mybir.dt.float32
    f32r = mybir.dt.float32r
    # dtype for matmul-feeding tensors. All of these stream 1 col/cycle on
    # the PE (plain fp32 runs at 1/4 rate), but 4-byte f32r caps at ~2.13GHz
    # effective on the moving-operand read path while 2-byte dtypes hit the
    # full 2.4GHz. f16 keeps 10 mantissa bits vs bf16's 8.
    mmdt = {"f32r": f32r, "bf16": mybir.dt.bfloat16, "f16": mybir.dt.float16}[MM_DT]
    two_byte = MM_DT in ("bf16", "f16")
    SILU = mybir.ActivationFunctionType.Silu

    nc = bacc.Bacc("TRN2", target_bir_lowering=False, debug=False)

    xp = nc.dram_tensor("xp", [IPC, 2, 128, PADN], mmdt, kind="ExternalInput")
    w1t = nc.dram_tensor("w1t", [128, 18 * 128], mmdt, kind="ExternalInput")
    w2t = nc.dram_tensor("w2t", [128, 18 * 128], mmdt, kind="ExternalInput")
    ab1 = nc.dram_tensor("ab1", [128, 2], f32, kind="ExternalInput")
    ab2 = nc.dram_tensor("ab2", [128, 4], f32, kind="ExternalInput")
    if two_byte:
        # residual correction plane: xlo = x - mmdt(x), so the shortcut adds
        # hi + lo = x to ~fp32 precision while matmuls read only hi.
        xlo = nc.dram_tensor("xlo", [IPC, 2, 128, H * W], mmdt, kind="ExternalInput")
    y = nc.dram_tensor("y", [IPC, 2, 128, H * W], f32, kind="ExternalOutput")

    TAPS = [(ky, kx) for ky in range(3) for kx in range(3)]

    with tile.TileContext(nc) as tc:
        with (
            tc.tile_pool(name="consts", bufs=1) as cpool,
            tc.tile_pool(name="xin", bufs=2) as xpool,
            tc.tile_pool(name="hbuf", bufs=1) as hpool,
            tc.tile_pool(name="stage", bufs=4) as spool,
            tc.tile_pool(name="outs", bufs=4) as opool,
            tc.tile_pool(name="ps", bufs=4, space=bass.MemorySpace.PSUM) as pspool,
        ):
            # weights/consts/border-zeros on the vector DGE queue so the
            # x-image loads (sync queue) aren't stuck behind them.
            W1 = cpool.tile([128, 18 * 128], mmdt, tag="W1")
            nc.scalar.dma_start(W1[:], w1t[:, :])
            AB1 = cpool.tile([128, 2], f32, tag="AB1")
            nc.scalar.dma_start(AB1[:], ab1[:, :])
            AB2 = cpool.tile([128, 4], f32, tag="AB2")
            nc.scalar.dma_start(AB2[:], ab2[:, :])
            W2 = cpool.tile([128, 18 * 128], mmdt, tag="W2")
            nc.scalar.dma_start(W2[:], w2t[:, :])

            # h ping-pong tiles; zero the padding border once, interior is
            # fully rewritten every image. Writes go through ScalarE's Silu
            # with scale=0 (silu(0)=0): the only compute op proven to emit
            # float32r, and it avoids a 4-byte-packet DMA storm.
            zrow = cpool.tile([128, WP], f32, tag="zrow")
            nc.gpsimd.memset(zrow[:], 0.0)
            hts = []
            for j in range(2):
                ht = hpool.tile([128, PADN], mmdt, tag=f"h{j}")
                hv = ht[:].rearrange("p (r c) -> p r c", r=HP, c=WP)
                for dst, n in (
                    (hv[:, 0, :], WP),
                    (hv[:, HP - 1, :], WP),
                    (hv[:, 1 : HP - 1, 0:1], HP - 2),
                    (hv[:, 1 : HP - 1, WP - 1 : WP], HP - 2),
                ):
                    nc.scalar.activation(dst, zrow[:, :n], SILU, bias=0.0, scale=0.0)
                hts.append(ht)

            # padded-row chunks: rowblock r needs padded rows [8r, 8r+10)
            XCHUNKS = [(0, 18), (18, 34), (34, 50), (50, HP)]

            for img in range(IPC):
                X = xpool.tile([128, 2 * PADN], mmdt, tag="X")
                for r0, r1 in XCHUNKS:
                    for blk in range(2):
                        nc.sync.dma_start(
                            X[:, blk * PADN + r0 * WP : blk * PADN + r1 * WP],
                            xp[img, blk, :, r0 * WP : r1 * WP],
                        )
                Xv = X[:].rearrange("p (b r c) -> p b r c", b=2, r=HP, c=WP)
                if two_byte:
                    XL = xpool.tile([128, 2 * H * W], mmdt, tag="XL")
                    for blk in range(2):
                        nc.gpsimd.dma_start(
                            XL[:, blk * H * W : (blk + 1) * H * W], xlo[img, blk, :, :]
                        )
                    XLv = XL[:].rearrange("p (b n) -> p b n", b=2)
                ht = hts[img % 2]
                hv = ht[:].rearrange("p (r c) -> p r c", r=HP, c=WP)

                # conv1: 256 -> 128, BN+SiLU into padded h interior
                for r in range(NRB):
                    ps1 = pspool.tile([128, NMM], f32, tag="ps1")
                    k = 0
                    for t, (ky, kx) in enumerate(TAPS):
                        for blk in range(2):
                            rhs = Xv[:, blk, RB * r + ky : RB * r + ky + RB, kx : kx + W]
                            lhsT = W1[:, (t * 2 + blk) * 128 : (t * 2 + blk + 1) * 128]
                            nc.tensor.matmul(
                                ps1[:],
                                lhsT,
                                rhs,
                                start=(k == 0),
                                stop=(k == 17),
                            )
                            k += 1
                    nc.scalar.activation(
                        hv[:, 1 + RB * r : 1 + RB * r + RB, 1 : 1 + W],
                        ps1[:],
                        SILU,
                        bias=AB1[:, 1:2],
                        scale=AB1[:, 0:1],
                    )

                # conv2: 128 -> 256 (two cout blocks), BN+SiLU, +residual, store
                for r in range(NRB):
                    for cb in range(2):
                        ps2 = pspool.tile([128, NMM], f32, tag="ps2")
                        for t, (ky, kx) in enumerate(TAPS):
                            rhs = hv[:, RB * r + ky : RB * r + ky + RB, kx : kx + W]
                            lhsT = W2[:, (t * 2 + cb) * 128 : (t * 2 + cb + 1) * 128]
                            nc.tensor.matmul(
                                ps2[:],
                                lhsT,
                                rhs,
                                start=(t == 0),
                                stop=(t == 8),
                            )
                        st = spool.tile([128, NMM], f32, tag="st")
                        nc.scalar.activation(
                            st[:],
                            ps2[:],
                            SILU,
                            bias=AB2[:, 2 * cb + 1 : 2 * cb + 2],
                            scale=AB2[:, 2 * cb : 2 * cb + 1],
                        )
                        ot = opool.tile([128, NMM], f32, tag="ot")
                        xres = Xv[:, cb, 1 + RB * r : 1 + RB * r + RB, 1 : 1 + W]
                        if MM_DT == "f32r":
                            xres = xres.bitcast(f32)
                        nc.vector.tensor_add(ot[:], st[:], xres)
                        if two_byte:
                            nc.vector.tensor_add(
                                ot[:], ot[:], XLv[:, cb, r * NMM : (r + 1) * NMM]
                            )
                        nc.gpsimd.dma_start(
                            y[img, cb, :, r * NMM : (r + 1) * NMM], ot[:]
                        )

    nc.compile()
    _CACHE["nc"] = nc
    return nc


def _quant_ternary(w):
    """Match jnp: s = max(median(|w|), Q_EPS); t = clip(round(w/s), -1, 1)."""
    w = np.asarray(w, np.float32)
    s = np.float32(np.median(np.abs(w)))
    s = np.maximum(s, np.float32(Q_EPS))
    t = np.clip(np.round(w / s), np.float32(-1.0), np.float32(1.0)).astype(np.float32)
    return s, t


def prepare_inputs(x, w1, g1, b1, m1, v1, w2, g2, b2, m2, v2):
    """Host-side prep: quantize+fold weights, pad x, build per-core in_maps."""
    x = np.asarray(x, np.float32)

    s1, t1 = _quant_ternary(w1)
    s2, t2 = _quant_ternary(w2)

    inv1 = np.asarray(g1, np.float32) / np.sqrt(np.asarray(v1, np.float32) + np.float32(BN_EPS))
    a1 = (s1 * inv1).astype(np.float32)  # [HID]
    c1 = (np.asarray(b1, np.float32) - np.asarray(m1, np.float32) * inv1).astype(np.float32)
    inv2 = np.asarray(g2, np.float32) / np.sqrt(np.asarray(v2, np.float32) + np.float32(BN_EPS))
    a2 = (s2 * inv2).astype(np.float32)  # [C]
    c2 = (np.asarray(b2, np.float32) - np.asarray(m2, np.float32) * inv2).astype(np.float32)

    ab1 = np.stack([a1, c1], axis=1).astype(np.float32)  # [128, 2]
    a2b = a2.reshape(2, 128)
    c2b = c2.reshape(2, 128)
    ab2 = np.stack([a2b[0], c2b[0], a2b[1], c2b[1]], axis=1).astype(np.float32)  # [128,4]

    # lhsT layouts: [cin128, ((ky*3+kx)*2 + blk)*128 + cout]
    w1t = (
        t1.reshape(HID, 2, 128, 3, 3).transpose(2, 3, 4, 1, 0).reshape(128, 18 * 128)
    ).astype(np.float32).copy()
    w2t = (
        t2.reshape(2, 128, HID, 3, 3).transpose(2, 3, 4, 0, 1).reshape(128, 18 * 128)
    ).astype(np.float32).copy()

    if MM_DT == "bf16":
        import ml_dtypes

        mmnp = np.dtype(ml_dtypes.bfloat16)
    elif MM_DT == "f16":
        mmnp = np.dtype(np.float16)
    else:
        mmnp = np.dtype(np.float32)
    two_byte = MM_DT in ("bf16", "f16")
    w1t = w1t.astype(mmnp)
    w2t = w2t.astype(mmnp)

    xhi = x.astype(mmnp)
    xpad = np.zeros((B, C, HP, WP), mmnp)
    xpad[:, :, 1 : 1 + H, 1 : 1 + W] = xhi
    xp = xpad.reshape(NCORES, IPC, 2, 128, PADN)
    if two_byte:
        xlo = (x - xhi.astype(np.float32)).astype(mmnp)
        xlo = xlo.reshape(NCORES, IPC, 2, 128, H * W)

    in_maps = []
    for c in range(NCORES):
        m = {
            "xp": np.ascontiguousarray(xp[c]),
            "w1t": w1t,
            "w2t": w2t,
            "ab1": ab1,
            "ab2": ab2,
        }
        if two_byte:
            m["xlo"] = np.ascontiguousarray(xlo[c])
        in_maps.append(m)
    return in_maps


def assemble_output(per_core_results):
    ys = np.stack([r["y"] for r in per_core_results])  # [8, IPC, 2, 128, H*W]
    return ys.reshape(B, C, H, W).astype(np.float32)


def run_spmd(in_maps, **kwargs):
    from concourse.bass_utils import run_bass_kernel_spmd

    nc = _build_nc()
    return run_bass_kernel_spmd(nc, in_maps, core_ids=list(range(NCORES)), **kwargs)


def kernel(**inputs):
    in_maps = prepare_inputs(**inputs)
    res = run_spmd(in_maps)
    return assemble_output(res.results)
